# revision 15
# baseline (speedup 1.0000x reference)
"""Deformable conv block (3x3 offset conv -> 3x3 deformable group conv), 8x trn2.

v3.3: like v3.1/v3.2 (3x3 tent window + host patch of ~200 outliers, 96
q-rows, SBUF q, fp16 fold tree) plus:
  - shift-major PSUM layout for T-builds: each PSUM bank holds slots of ONE
    column shift, so a T-build is 6 matmuls + 7 permuting scalar drains
    (was 17 matmuls + bank drains). The drain's strided write produces the
    ky-major slot-major SBUF layout the combine needs.
  - offset conv fused with the tent selector: rep ( one-hot [18,96] ) is
    folded into the conv weights host-side (cwy/cwx [72, 9*96]), so the
    9-tap conv accumulates directly into the [96, W] tent-argument PSUM.
    Saves a PSUM bank and the [18,W] intermediate.
  - outlier detection from the returned ty/tx tent maps (center-tent == 0
    <=> |offset| >= 1); exact offsets for flagged pixels are recomputed on
    the host from offset_feat.
"""

import numpy as np
from contextlib import ExitStack

import concourse.bass as bass
import concourse.tile as tile
from concourse import bacc, mybir
from concourse import bass_utils
from concourse.bass import AP

# Problem constants
B, C, O, H, W = 2, 72, 72, 180, 320
NK = 9
OC = 18
PADC = 2
WP = W + 2 * PADC     # 324
NQ = 4
RS = H // NQ          # 45
HALO = 2
RSP = RS + 2 * HALO   # 49
NPIX_I = RSP * WP
FROWS = RS + 2
NPIX_F = FROWS * WP
N_CORES = 8

F32 = mybir.dt.float32
F16 = mybir.dt.float16

# Per tap-row ky, the 9 (kx, v) pairs sorted by column shift s = kx-1 + v-1.
SLOT9 = sorted(((kx - 1 + v - 1, kx, v) for kx in range(3) for v in range(3)))
NSPK = 9              # real slots per ky in the SBUF T-slab
NSLOT = 27            # no pads: a-groups read exactly the real slots
NQR = 96
NQU = 81              # q rows actually used (5 a-groups, no padding)

# a-groups: row offset a = (ky-1)+(u-1); (a, qbase, t0=T-slot start, n)
AGROUPS = [(-2, 0, 0, 9), (-1, 9, 0, 18), (0, 27, 0, 27),
           (1, 54, 9, 18), (2, 72, 18, 9)]

# PSUM banks, shift-major: (shift, [ (ky, slot9-idx) | None ] * 6)
_bys = {s: [] for s in (-2, -1, 0, 1, 2)}
for _ky in range(3):
    for _i, (_s, _kx, _v) in enumerate(SLOT9):
        _bys[_s].append((_ky, _i))
PBANKS = [
    (-2, _bys[-2] + [None] * 3),
    (-1, _bys[-1]),
    (0, _bys[0][:6]),
    (0, _bys[0][6:] + [None] * 3),
    (1, _bys[1]),
    (2, _bys[2] + [None] * 3),
]

# drains: (pool_idx, psum_off, ncols, out_dims, out_off)
DRAINS = [
    (0, 0, 216, [[648, 3], [1, 72]], 0),
    (0, 512, 432, [[648, 3], [72, 2], [1, 72]], 1 * 72),
    (0, 1024, 432, [[648, 2], [72, 3], [1, 72]], 3 * 72),
    (1, 0, 216, [[72, 3], [1, 72]], 21 * 72),
    (1, 512, 432, [[648, 3], [72, 2], [1, 72]], 6 * 72),
    (1, 1024, 216, [[648, 3], [1, 72]], 8 * 72),
]

CT01 = [(PADC, 128), (PADC + 128, 128)]
C2 = PADC + 256
PAIRS = [(2 * i, 2 * i + 1) for i in range(22)] + [(43, 44)]


def _qrow_tables():
    """repy/repx one-hot [18, 96], biases, and per-tap center-row indices."""
    repy = np.zeros((OC, NQR), np.float32)
    repx = np.zeros((OC, NQR), np.float32)
    biasu = np.full(NQR, -3.0, np.float32)
    biasv = np.full(NQR, -3.0, np.float32)
    rowy = [None] * NK
    rowx = [None] * NK
    for (a, qb, t0, n) in AGROUPS:
        for j in range(n):
            slot = t0 + j
            if slot >= 27:
                continue
            ky, i = slot // NSPK, slot % NSPK
            u = a - (ky - 1) + 1
            if u < 0 or u > 2:
                continue
            s, kx, v = SLOT9[i]
            k = ky * 3 + kx
            row = qb + j
            repy[2 * k, row] = 1.0
            repx[2 * k + 1, row] = 1.0
            biasu[row] = -(u - 1)
            biasv[row] = -(v - 1)
            if u == 1 and rowy[k] is None:
                rowy[k] = row
            if v == 1 and rowx[k] is None:
                rowx[k] = row
    return repy, repx, biasu, biasv, rowy, rowx


_REPY, _REPX, _BIASU, _BIASV, ROWY, ROWX = _qrow_tables()


def build_module():
    nc = bacc.Bacc("TRN2", target_bir_lowering=False, debug=False,
                   num_devices=N_CORES)

    img_d = nc.dram_tensor("img", [C, NPIX_I], F16, kind="ExternalInput")
    feat_d = nc.dram_tensor("feat", [C, NPIX_F], F16, kind="ExternalInput")
    wts_d = nc.dram_tensor("wts", [C, 36 * O], F16, kind="ExternalInput")
    offw_d = nc.dram_tensor("offw", [C, 9 * OC], F16, kind="ExternalInput")
    offb_d = nc.dram_tensor("offb", [OC, 1], F32, kind="ExternalInput")
    repy_d = nc.dram_tensor("repy", [OC, NQR], F16, kind="ExternalInput")
    repx_d = nc.dram_tensor("repx", [OC, NQR], F16, kind="ExternalInput")
    biasu_d = nc.dram_tensor("biasu", [NQR, 1], F32, kind="ExternalInput")
    biasv_d = nc.dram_tensor("biasv", [NQR, 1], F32, kind="ExternalInput")
    dmat_d = nc.dram_tensor("dmat", [NQR, 2 * NQR], F16, kind="ExternalInput")
    outh_d = nc.dram_tensor("outh", [RS * W, 5 * O], F16,
                            kind="ExternalOutput")
    outc_d = nc.dram_tensor("outc", [RS * W, O], F16, kind="ExternalOutput")
    tya_d = nc.dram_tensor("tya", [NQR, RS * W], F16, kind="ExternalOutput")
    txa_d = nc.dram_tensor("txa", [NQR, RS * W], F16, kind="ExternalOutput")

    with tile.TileContext(nc) as tc, ExitStack() as ctx:
        const = ctx.enter_context(tc.tile_pool(name="const", bufs=1))
        big = ctx.enter_context(tc.tile_pool(name="big", bufs=1))
        featp = ctx.enter_context(tc.tile_pool(name="featp", bufs=6))
        sc = ctx.enter_context(tc.tile_pool(name="sc", bufs=3))
        q2p = ctx.enter_context(tc.tile_pool(name="q2p", bufs=4))
        tpool = ctx.enter_context(tc.tile_pool(name="tpool", bufs=6))
        apool = ctx.enter_context(tc.tile_pool(name="apool", bufs=3))
        rpool = ctx.enter_context(tc.tile_pool(name="rpool", bufs=2))
        ps_rep = ctx.enter_context(
            tc.tile_pool(name="ps_rep", bufs=1, space="PSUM"))
        ps_mix = ctx.enter_context(
            tc.tile_pool(name="ps_mix", bufs=1, space="PSUM"))
        ps_Ta = ctx.enter_context(
            tc.tile_pool(name="ps_Ta", bufs=1, space="PSUM"))
        ps_Tb = ctx.enter_context(
            tc.tile_pool(name="ps_Tb", bufs=1, space="PSUM"))

        wts = const.tile([C, 36 * O], F16)
        nc.sync.dma_start(wts[:], wts_d[:])
        offw = const.tile([C, 9 * OC], F16)
        nc.sync.dma_start(offw[:], offw_d[:])
        offb = const.tile([OC, 1], F32)
        nc.sync.dma_start(offb[:], offb_d[:])
        repy = const.tile([OC, NQR], F16)
        nc.sync.dma_start(repy[:], repy_d[:])
        repx = const.tile([OC, NQR], F16)
        nc.sync.dma_start(repx[:], repx_d[:])
        biasu = const.tile([NQR, 1], F32)
        nc.sync.dma_start(biasu[:], biasu_d[:])
        biasv = const.tile([NQR, 1], F32)
        nc.sync.dma_start(biasv[:], biasv_d[:])
        dmat = const.tile([NQR, 2 * NQR], F16)
        nc.sync.dma_start(dmat[:], dmat_d[:])
        imgh = big.tile([C, NPIX_I], F16)
        nc.sync.dma_start(imgh[:], img_d[:])

        feat_rows = {}
        q2_tiles = {}
        q2pair = {}
        t_tiles = [{}, {}]
        t2_tiles = {}

        def load_feat(fr):
            t = featp.tile([C, WP], F16, tag="featrow")
            nc.sync.dma_start(t[:], feat_d[:, (fr + 1) * WP:(fr + 2) * WP])
            feat_rows[fr] = t

        def phase1(r):
            mix = ps_mix.tile([128, 512], F32, tag="mix")
            for t in range(9):
                dr_, dc = t // 3 - 1, t % 3 - 1
                frow = feat_rows[r + dr_]
                nc.tensor.matmul(
                    mix[:OC, 0:W], offw[:, t * OC:(t + 1) * OC],
                    frow[:, PADC + dc: PADC + dc + W],
                    start=(t == 0), stop=(t == 8))
            offs = sc.tile([OC, W], F16, tag="offs")
            nc.vector.tensor_scalar(
                out=offs[:], in0=mix[:OC, 0:W], scalar1=offb[:], scalar2=None,
                op0=mybir.AluOpType.add)
            ta = {}
            for (rep, bia, nm, od) in ((repy, biasu, "ty", tya_d),
                                       (repx, biasv, "tx", txa_d)):
                pr = ps_rep.tile([NQR, W], F32, tag="pr")
                nc.tensor.matmul(pr[:, :], rep[:, :], offs[:],
                                 start=True, stop=True)
                tt_ = sc.tile([NQR, W], F16, tag=nm)
                nc.scalar.activation(
                    tt_[:, :], pr[:, :], mybir.ActivationFunctionType.Abs,
                    bias=bia[:], scale=1.0)
                nc.scalar.activation(
                    tt_[:, :], tt_[:, :], mybir.ActivationFunctionType.Relu,
                    bias=1.0, scale=-1.0)
                nc.sync.dma_start(od[:, r * W:(r + 1) * W], tt_[:])
                ta[nm] = tt_
            qrow = sc.tile([NQR, W], F16, tag="qrow")
            nc.gpsimd.tensor_tensor(out=qrow[:], in0=ta["ty"][:],
                                    in1=ta["tx"][:], op=mybir.AluOpType.mult)

            for ct, (c0, tw) in enumerate(CT01):
                nc.tensor.matmul(
                    mix[:tw, W:W + 2 * NQR],
                    qrow[:, c0 - PADC: c0 - PADC + tw],
                    dmat[:, :], start=True, stop=True)
                qt = q2p.tile([128, 2 * NQR], F16, tag=f"q2_{ct}")
                nc.scalar.copy(qt[:tw, :], mix[:tw, W:W + 2 * NQR])
                q2_tiles[(r, ct)] = qt
            for p, (ra, rb) in enumerate(PAIRS):
                if r not in (ra, rb):
                    continue
                par = 0 if r == ra else 1
                if par == 0:
                    q2pair[p] = q2p.tile([128, 2 * NQR], F16, tag="q2_2",
                                         name=f"q2pair{p}")
                nc.tensor.matmul(
                    mix[par * 64:(par + 1) * 64, W:W + 2 * NQR],
                    qrow[:, 256:320],
                    dmat[:, :], start=True, stop=True)
                nc.scalar.copy(
                    q2pair[p][par * 64:(par + 1) * 64, :],
                    mix[par * 64:(par + 1) * 64, W:W + 2 * NQR])

        def _alloc_pts():
            pta = ps_Ta.tile([128, 3 * 512], F32, tag="pTa", name="pta")
            ptb = ps_Tb.tile([128, 3 * 512], F32, tag="pTb", name="ptb")
            return (pta, ptb)

        def _drains(pts, tsb, np_):
            ta = tsb[:np_, :]
            for (pi, psoff, ncols, odims, ooff) in DRAINS:
                ps = pts[pi][:np_, :]
                inap = AP(ps.tensor, ps.offset + psoff,
                          [ps.ap[0], [1, ncols]])
                outap = AP(ta.tensor, ta.offset + ooff, [ta.ap[0]] + odims)
                nc.scalar.copy(outap, inap)

        def build_T(ct, rp):
            c0, tw = CT01[ct]
            base = (rp + HALO) * WP + c0
            pts = _alloc_pts()
            for b, (s, slots) in enumerate(PBANKS):
                pT = pts[b // 3]
                nc.tensor.matmul(
                    pT[:tw, (b % 3) * 512: (b % 3) * 512 + 432],
                    imgh[:, base + s: base + s + tw],
                    wts[:, b * 6 * O: (b * 6 + 6) * O],
                    start=True, stop=True)
            tsb = tpool.tile([128, NSLOT * O], F16, tag=f"tsb_{ct}")
            _drains(pts, tsb, tw)
            t_tiles[ct][rp] = tsb

        def build_T2(bs):
            pts = _alloc_pts()
            for half in (0, 1):
                base = (bs + half + HALO) * WP + C2
                for b, (s, slots) in enumerate(PBANKS):
                    pT = pts[b // 3]
                    nc.tensor.matmul(
                        pT[half * 64:(half + 1) * 64,
                           (b % 3) * 512: (b % 3) * 512 + 432],
                        imgh[:, base + s: base + s + 64],
                        wts[:, b * 6 * O: (b * 6 + 6) * O],
                        start=True, stop=True)
            tsb = tpool.tile([128, NSLOT * O], F16, tag="tsb_2")
            _drains(pts, tsb, 128)
            t2_tiles[bs] = tsb

        def combine(tw, slabs, q2t, dmas):
            prod = apool.tile([128, NQU * O], F16, tag="prod")
            pa = prod[:tw, :]
            qa = q2t[:tw, :]
            for (a, qb, t0, n) in AGROUPS:
                ts = slabs[a][:tw, :]
                in0 = AP(ts.tensor, ts.offset + t0 * O,
                         [ts.ap[0], [O, n], [2, O // 2], [1, 2]])
                in1 = AP(qa.tensor, qa.offset + 2 * qb,
                         [qa.ap[0], [2, n], [0, O // 2], [1, 2]])
                outp = AP(pa.tensor, pa.offset + qb * O,
                          [pa.ap[0], [O, n], [2, O // 2], [1, 2]])
                nc.vector.tensor_tensor(out=outp, in0=in0, in1=in1,
                                        op=mybir.AluOpType.mult)

            def fold(eng, src, n_el, dst):
                half = n_el // 2
                eng.tensor_tensor(
                    out=dst[:tw, 0:half], in0=src[:tw, 0:half],
                    in1=src[:tw, half:n_el], op=mybir.AluOpType.add)

            # tree over slots 0..79; slot 80 rides to the host as a carry
            h40 = rpool.tile([128, 40 * O], F16, tag="h40")
            fold(nc.vector, prod, 80 * O, h40)
            h20 = rpool.tile([128, 20 * O], F16, tag="h20")
            fold(nc.gpsimd, h40, 40 * O, h20)
            h10 = rpool.tile([128, 10 * O], F16, tag="h10")
            fold(nc.vector, h20, 20 * O, h10)
            h5 = rpool.tile([128, 5 * O], F16, tag="h5")
            fold(nc.vector, h10, 10 * O, h5)
            for (orow, plo, phi) in dmas:
                nc.sync.dma_start(outh_d[orow:orow + (phi - plo), :],
                                  h5[plo:phi, :])
                nc.sync.dma_start(outc_d[orow:orow + (phi - plo), :],
                                  prod[plo:phi, 80 * O:81 * O])

        # ---------------- prologue ----------------
        for fr in range(-1, 4):
            load_feat(fr)
        phase1(0)
        phase1(1)
        for ct in range(2):
            for rp in range(-2, 2):
                build_T(ct, rp)
        for bs in range(-2, 1):
            build_T2(bs)

        # ---------------- main loop ----------------
        for r in range(RS):
            if r <= RS - 3:
                load_feat(r + 3)
                phase1(r + 2)
            build_T(0, r + 2)
            build_T(1, r + 2)
            build_T2(r + 1)
            for ct, (c0, tw) in enumerate(CT01):
                slabs = {a: t_tiles[ct][r + a] for a in range(-2, 3)}
                combine(tw, slabs, q2_tiles[(r, ct)],
                        [(r * W + (c0 - PADC), 0, tw)])
            pidx = None
            if r % 2 == 1 and r <= 43:
                pidx = (r - 1) // 2
            elif r == RS - 1:
                pidx = len(PAIRS) - 1
            if pidx is not None:
                ra = PAIRS[pidx][0]
                slabs = {a: t2_tiles[ra + a] for a in range(-2, 3)}
                combine(128, slabs, q2pair[pidx],
                        [(ra * W + 256, 0, 64), ((ra + 1) * W + 256, 64, 128)])

    nc.compile()
    return nc


# ------------------------- host side -------------------------

_nc_cache = [None]


def _get_nc():
    if _nc_cache[0] is None:
        _nc_cache[0] = build_module()
    return _nc_cache[0]


def _consts(weight, off_w, off_b):
    wk = np.zeros((NK, C, O), np.float32)
    for g in range(9):
        for og in range(8):
            for cg in range(8):
                for k in range(NK):
                    wk[k, g * 8 + cg, g * 8 + og] = weight[
                        g * 8 + og, cg, k // 3, k % 3]
    # wts columns follow the shift-major PSUM slot order
    wts = np.zeros((C, 36 * O), np.float16)
    for b, (s, slots) in enumerate(PBANKS):
        for w, ent in enumerate(slots):
            if ent is None:
                continue
            ky, i = ent
            _, kx, v = SLOT9[i]
            k = ky * 3 + kx
            m = b * 6 + w
            wts[:, m * O:(m + 1) * O] = wk[k].astype(np.float16)

    offw = np.zeros((C, 9 * OC), np.float16)
    for t in range(9):
        offw[:, t * OC:(t + 1) * OC] = off_w[:, :, t // 3, t % 3].T
    biasu = _BIASU.reshape(NQR, 1).astype(np.float32)
    biasv = _BIASV.reshape(NQR, 1).astype(np.float32)

    dmat = np.zeros((NQR, 2 * NQR), np.float16)
    for j in range(NQR):
        dmat[j, 2 * j] = 1.0
        dmat[j, 2 * j + 1] = 1.0
    return {
        "wts": wts, "offw": offw,
        "offb": off_b.reshape(OC, 1).astype(np.float32),
        "repy": _REPY.astype(np.float16), "repx": _REPX.astype(np.float16),
        "biasu": biasu, "biasv": biasv, "dmat": dmat,
    }


def _slab(x_b, halo, rows):
    out = []
    for q in range(NQ):
        s = np.zeros((C, rows, WP), np.float16)
        lo, hi = q * RS - halo, q * RS + RS + halo
        clo, chi = max(lo, 0), min(hi, H)
        s[:, clo - lo: clo - lo + (chi - clo), PADC:PADC + W] = x_b[:, clo:chi]
        out.append(np.ascontiguousarray(s.reshape(C, rows * WP)))
    return out


def _patch_outliers(out, inp, offset_feat, weight, off_w, off_b, tya, txa):
    """Fix (tap,pixel) events where |offset| >= 1 (center tent == 0)."""
    featp = np.pad(offset_feat, ((0, 0), (0, 0), (1, 1), (1, 1)))
    evs = []
    for k in range(NK):
        fy = tya[:, ROWY[k]] < 1e-3
        fx = txa[:, ROWX[k]] < 1e-3
        bs, ys, xs = np.nonzero(fy | fx)
        for b, y, x in zip(bs, ys, xs):
            evs.append((b, k, y, x))
    g = np.arange(O) // 8
    for (b, k, y, x) in evs:
        ky, kx = k // 3, k % 3
        nb = featp[b, :, y:y + 3, x:x + 3]
        dyv = float((off_w[2 * k] * nb).sum() + off_b[2 * k])
        dxv = float((off_w[2 * k + 1] * nb).sum() + off_b[2 * k + 1])
        py = y + ky - 1 + dyv
        px = x + kx - 1 + dxv
        y0, x0 = int(np.floor(py)), int(np.floor(px))
        wy1, wx1 = py - y0, px - x0
        s_true = np.zeros(C, np.float32)
        for yi, wy in ((y0, 1.0 - wy1), (y0 + 1, wy1)):
            for xi, wx in ((x0, 1.0 - wx1), (x0 + 1, wx1)):
                if 0 <= yi < H and 0 <= xi < W and wy * wx != 0.0:
                    s_true += (wy * wx) * inp[b, :, yi, xi]
        s_kern = np.zeros(C, np.float32)
        for u in (-1, 0, 1):
            tyv = max(0.0, 1.0 - abs(dyv - u))
            if tyv == 0.0:
                continue
            for v in (-1, 0, 1):
                txv = max(0.0, 1.0 - abs(dxv - v))
                if txv == 0.0:
                    continue
                yy, xx = y + ky - 1 + u, x + kx - 1 + v
                if 0 <= yy < H and 0 <= xx < W:
                    s_kern += (tyv * txv) * inp[b, :, yy, xx]
        delta = (s_true - s_kern).reshape(9, 8)
        corr = (weight[:, :, ky, kx] * delta[g]).sum(axis=1)
        out[b, :, y, x] += corr


def kernel(input, offset_feat, weight, off_w, off_b):
    input = np.asarray(input, np.float32)
    offset_feat = np.asarray(offset_feat, np.float32)
    weight = np.asarray(weight, np.float32)
    off_w = np.asarray(off_w, np.float32)
    off_b = np.asarray(off_b, np.float32)

    nc = _get_nc()
    consts = _consts(weight, off_w, off_b)
    in_maps = []
    for b in range(B):
        imgs = _slab(input[b], HALO, RSP)
        feats = _slab(offset_feat[b], 1, FROWS)
        for q in range(NQ):
            m = dict(consts)
            m["img"] = imgs[q]
            m["feat"] = feats[q]
            in_maps.append(m)

    res = bass_utils.run_bass_kernel_spmd(
        nc, in_maps, core_ids=list(range(N_CORES)))

    out = np.empty((B, O, H, W), np.float32)
    tya = np.empty((B, NQR, H, W), np.float32)
    txa = np.empty((B, NQR, H, W), np.float32)
    for ci in range(N_CORES):
        b, q = ci // NQ, ci % NQ
        oh = res.results[ci]["outh"].reshape(RS, W, 5, O).astype(np.float32)
        ocr = res.results[ci]["outc"].reshape(RS, W, O).astype(np.float32)
        o = oh.sum(axis=2) + ocr
        out[b, :, q * RS:(q + 1) * RS, :] = o.transpose(2, 0, 1)
        tya[b, :, q * RS:(q + 1) * RS, :] = (
            res.results[ci]["tya"].reshape(NQR, RS, W).astype(np.float32))
        txa[b, :, q * RS:(q + 1) * RS, :] = (
            res.results[ci]["txa"].reshape(NQR, RS, W).astype(np.float32))

    _patch_outliers(out, input, offset_feat, weight, off_w, off_b, tya, txa)
    return out


if __name__ == "__main__":
    import reference as ref
    inputs = {k: np.asarray(v) for k, v in ref.setup_inputs().items()}
    got = kernel(**inputs)
    print("out", got.shape, got.dtype)


# revision 17
# speedup vs baseline: 1.0127x; 1.0127x over previous
"""Deformable conv block (3x3 offset conv -> 3x3 deformable group conv), 8x trn2.

v3.3: like v3.1/v3.2 (3x3 tent window + host patch of ~200 outliers, 96
q-rows, SBUF q, fp16 fold tree) plus:
  - shift-major PSUM layout for T-builds: each PSUM bank holds slots of ONE
    column shift, so a T-build is 6 matmuls + 7 permuting scalar drains
    (was 17 matmuls + bank drains). The drain's strided write produces the
    ky-major slot-major SBUF layout the combine needs.
  - offset conv fused with the tent selector: rep ( one-hot [18,96] ) is
    folded into the conv weights host-side (cwy/cwx [72, 9*96]), so the
    9-tap conv accumulates directly into the [96, W] tent-argument PSUM.
    Saves a PSUM bank and the [18,W] intermediate.
  - outlier detection from the returned ty/tx tent maps (center-tent == 0
    <=> |offset| >= 1); exact offsets for flagged pixels are recomputed on
    the host from offset_feat.
"""

import numpy as np
from contextlib import ExitStack

import concourse.bass as bass
import concourse.tile as tile
from concourse import bacc, mybir
from concourse import bass_utils
from concourse.bass import AP

# Problem constants
B, C, O, H, W = 2, 72, 72, 180, 320
NK = 9
OC = 18
PADC = 2
WP = W + 2 * PADC     # 324
NQ = 4
RS = H // NQ          # 45
HALO = 2
RSP = RS + 2 * HALO   # 49
NPIX_I = RSP * WP
FROWS = RS + 2
NPIX_F = FROWS * WP
N_CORES = 8

F32 = mybir.dt.float32
F16 = mybir.dt.float16

# Per tap-row ky, the 9 (kx, v) pairs sorted by column shift s = kx-1 + v-1.
SLOT9 = sorted(((kx - 1 + v - 1, kx, v) for kx in range(3) for v in range(3)))
NSPK = 9              # real slots per ky in the SBUF T-slab
NSLOT = 27            # no pads: a-groups read exactly the real slots
NQR = 96
NQU = 81              # q rows actually used (5 a-groups, no padding)

# a-groups: row offset a = (ky-1)+(u-1); (a, qbase, t0=T-slot start, n)
AGROUPS = [(-2, 0, 0, 9), (-1, 9, 0, 18), (0, 27, 0, 27),
           (1, 54, 9, 18), (2, 72, 18, 9)]

# PSUM banks, shift-major: (shift, [ (ky, slot9-idx) | None ] * 6)
_bys = {s: [] for s in (-2, -1, 0, 1, 2)}
for _ky in range(3):
    for _i, (_s, _kx, _v) in enumerate(SLOT9):
        _bys[_s].append((_ky, _i))
PBANKS = [
    (-2, _bys[-2] + [None] * 3),
    (-1, _bys[-1]),
    (0, _bys[0][:6]),
    (0, _bys[0][6:] + [None] * 3),
    (1, _bys[1]),
    (2, _bys[2] + [None] * 3),
]

# drains: (pool_idx, psum_off, ncols, out_dims, out_off)
DRAINS = [
    (0, 0, 216, [[648, 3], [1, 72]], 0),
    (0, 512, 432, [[648, 3], [72, 2], [1, 72]], 1 * 72),
    (0, 1024, 432, [[648, 2], [72, 3], [1, 72]], 3 * 72),
    (1, 0, 216, [[72, 3], [1, 72]], 21 * 72),
    (1, 512, 432, [[648, 3], [72, 2], [1, 72]], 6 * 72),
    (1, 1024, 216, [[648, 3], [1, 72]], 8 * 72),
]

CT01 = [(PADC, 128), (PADC + 128, 128)]
C2 = PADC + 256
PAIRS = [(2 * i, 2 * i + 1) for i in range(22)] + [(43, 44)]


def _qrow_tables():
    """repy/repx one-hot [18, 96], biases, and per-tap center-row indices."""
    repy = np.zeros((OC, NQR), np.float32)
    repx = np.zeros((OC, NQR), np.float32)
    biasu = np.full(NQR, -3.0, np.float32)
    biasv = np.full(NQR, -3.0, np.float32)
    rowy = [None] * NK
    rowx = [None] * NK
    for (a, qb, t0, n) in AGROUPS:
        for j in range(n):
            slot = t0 + j
            if slot >= 27:
                continue
            ky, i = slot // NSPK, slot % NSPK
            u = a - (ky - 1) + 1
            if u < 0 or u > 2:
                continue
            s, kx, v = SLOT9[i]
            k = ky * 3 + kx
            row = qb + j
            repy[2 * k, row] = 1.0
            repx[2 * k + 1, row] = 1.0
            biasu[row] = -(u - 1)
            biasv[row] = -(v - 1)
            if u == 1 and rowy[k] is None:
                rowy[k] = row
            if v == 1 and rowx[k] is None:
                rowx[k] = row
    return repy, repx, biasu, biasv, rowy, rowx


_REPY, _REPX, _BIASU, _BIASV, ROWY, ROWX = _qrow_tables()


def build_module():
    nc = bacc.Bacc("TRN2", target_bir_lowering=False, debug=False,
                   num_devices=N_CORES)

    img_d = nc.dram_tensor("img", [C, NPIX_I], F16, kind="ExternalInput")
    feat_d = nc.dram_tensor("feat", [C, NPIX_F], F16, kind="ExternalInput")
    wts_d = nc.dram_tensor("wts", [C, 36 * O], F16, kind="ExternalInput")
    cwy_d = nc.dram_tensor("cwy", [C, 9 * NQR], F16, kind="ExternalInput")
    cwx_d = nc.dram_tensor("cwx", [C, 9 * NQR], F16, kind="ExternalInput")
    biasu_d = nc.dram_tensor("biasu", [NQR, 1], F32, kind="ExternalInput")
    biasv_d = nc.dram_tensor("biasv", [NQR, 1], F32, kind="ExternalInput")
    dmat_d = nc.dram_tensor("dmat", [NQR, 2 * NQR], F16, kind="ExternalInput")
    outh_d = nc.dram_tensor("outh", [RS * W, 5 * O], F16,
                            kind="ExternalOutput")
    outc_d = nc.dram_tensor("outc", [RS * W, O], F16, kind="ExternalOutput")
    tya_d = nc.dram_tensor("tya", [NQR, RS * W], F16, kind="ExternalOutput")
    txa_d = nc.dram_tensor("txa", [NQR, RS * W], F16, kind="ExternalOutput")

    with tile.TileContext(nc) as tc, ExitStack() as ctx:
        const = ctx.enter_context(tc.tile_pool(name="const", bufs=1))
        big = ctx.enter_context(tc.tile_pool(name="big", bufs=1))
        featp = ctx.enter_context(tc.tile_pool(name="featp", bufs=6))
        sc = ctx.enter_context(tc.tile_pool(name="sc", bufs=3))
        q2p = ctx.enter_context(tc.tile_pool(name="q2p", bufs=4))
        tpool = ctx.enter_context(tc.tile_pool(name="tpool", bufs=7))
        apool = ctx.enter_context(tc.tile_pool(name="apool", bufs=3))
        rpool = ctx.enter_context(tc.tile_pool(name="rpool", bufs=2))
        ps_rep = ctx.enter_context(
            tc.tile_pool(name="ps_rep", bufs=1, space="PSUM"))
        ps_tr = ctx.enter_context(
            tc.tile_pool(name="ps_tr", bufs=1, space="PSUM"))
        ps_Ta = ctx.enter_context(
            tc.tile_pool(name="ps_Ta", bufs=1, space="PSUM"))
        ps_Tb = ctx.enter_context(
            tc.tile_pool(name="ps_Tb", bufs=1, space="PSUM"))

        wts = const.tile([C, 36 * O], F16)
        nc.sync.dma_start(wts[:], wts_d[:])
        cwy = const.tile([C, 9 * NQR], F16)
        nc.sync.dma_start(cwy[:], cwy_d[:])
        cwx = const.tile([C, 9 * NQR], F16)
        nc.sync.dma_start(cwx[:], cwx_d[:])
        biasu = const.tile([NQR, 1], F32)
        nc.sync.dma_start(biasu[:], biasu_d[:])
        biasv = const.tile([NQR, 1], F32)
        nc.sync.dma_start(biasv[:], biasv_d[:])
        dmat = const.tile([NQR, 2 * NQR], F16)
        nc.sync.dma_start(dmat[:], dmat_d[:])
        imgh = big.tile([C, NPIX_I], F16)
        nc.sync.dma_start(imgh[:], img_d[:])

        feat_rows = {}
        q2_tiles = {}
        q2pair = {}
        t_tiles = [{}, {}]
        t2_tiles = {}

        def load_feat(fr):
            t = featp.tile([C, WP], F16, tag="featrow")
            nc.sync.dma_start(t[:], feat_d[:, (fr + 1) * WP:(fr + 2) * WP])
            feat_rows[fr] = t

        def phase1(r):
            ta = {}
            for (cw, bia, nm, od) in ((cwy, biasu, "ty", tya_d),
                                      (cwx, biasv, "tx", txa_d)):
                pr = ps_rep.tile([NQR, W], F32, tag="pr")
                for t in range(9):
                    dr_, dc = t // 3 - 1, t % 3 - 1
                    frow = feat_rows[r + dr_]
                    nc.tensor.matmul(
                        pr[:, :], cw[:, t * NQR:(t + 1) * NQR],
                        frow[:, PADC + dc: PADC + dc + W],
                        start=(t == 0), stop=(t == 8))
                tt_ = sc.tile([NQR, W], F16, tag=nm)
                nc.scalar.activation(
                    tt_[:, :], pr[:, :], mybir.ActivationFunctionType.Abs,
                    bias=bia[:], scale=1.0)
                nc.scalar.activation(
                    tt_[:, :], tt_[:, :], mybir.ActivationFunctionType.Relu,
                    bias=1.0, scale=-1.0)
                nc.sync.dma_start(od[:, r * W:(r + 1) * W], tt_[:])
                ta[nm] = tt_
            qrow = sc.tile([NQR, W], F16, tag="qrow")
            nc.gpsimd.tensor_tensor(out=qrow[:], in0=ta["ty"][:],
                                    in1=ta["tx"][:], op=mybir.AluOpType.mult)

            for ct, (c0, tw) in enumerate(CT01):
                ptq = ps_tr.tile([128, 2 * NQR], F32, tag="ptq")
                nc.tensor.matmul(
                    ptq[:tw, :], qrow[:, c0 - PADC: c0 - PADC + tw],
                    dmat[:, :], start=True, stop=True)
                qt = q2p.tile([128, 2 * NQR], F16, tag=f"q2_{ct}")
                nc.scalar.copy(qt[:tw, :], ptq[:tw, :])
                q2_tiles[(r, ct)] = qt
            for p, (ra, rb) in enumerate(PAIRS):
                if r not in (ra, rb):
                    continue
                par = 0 if r == ra else 1
                if par == 0:
                    q2pair[p] = q2p.tile([128, 2 * NQR], F16, tag="q2_2",
                                         name=f"q2pair{p}")
                ptq = ps_tr.tile([128, 2 * NQR], F32, tag="ptq")
                nc.tensor.matmul(
                    ptq[par * 64:(par + 1) * 64, :], qrow[:, 256:320],
                    dmat[:, :], start=True, stop=True)
                nc.scalar.copy(
                    q2pair[p][par * 64:(par + 1) * 64, :],
                    ptq[par * 64:(par + 1) * 64, :])

        def _alloc_pts():
            pta = ps_Ta.tile([128, 3 * 512], F32, tag="pTa", name="pta")
            ptb = ps_Tb.tile([128, 3 * 512], F32, tag="pTb", name="ptb")
            return (pta, ptb)

        def _drains(pts, tsb, np_):
            ta = tsb[:np_, :]
            for (pi, psoff, ncols, odims, ooff) in DRAINS:
                ps = pts[pi][:np_, :]
                inap = AP(ps.tensor, ps.offset + psoff,
                          [ps.ap[0], [1, ncols]])
                outap = AP(ta.tensor, ta.offset + ooff, [ta.ap[0]] + odims)
                nc.scalar.copy(outap, inap)

        def build_T(ct, rp):
            c0, tw = CT01[ct]
            base = (rp + HALO) * WP + c0
            pts = _alloc_pts()
            for b, (s, slots) in enumerate(PBANKS):
                pT = pts[b // 3]
                nc.tensor.matmul(
                    pT[:tw, (b % 3) * 512: (b % 3) * 512 + 432],
                    imgh[:, base + s: base + s + tw],
                    wts[:, b * 6 * O: (b * 6 + 6) * O],
                    start=True, stop=True)
            tsb = tpool.tile([128, NSLOT * O], F16, tag=f"tsb_{ct}")
            _drains(pts, tsb, tw)
            t_tiles[ct][rp] = tsb

        def build_T2(bs):
            pts = _alloc_pts()
            for half in (0, 1):
                base = (bs + half + HALO) * WP + C2
                for b, (s, slots) in enumerate(PBANKS):
                    pT = pts[b // 3]
                    nc.tensor.matmul(
                        pT[half * 64:(half + 1) * 64,
                           (b % 3) * 512: (b % 3) * 512 + 432],
                        imgh[:, base + s: base + s + 64],
                        wts[:, b * 6 * O: (b * 6 + 6) * O],
                        start=True, stop=True)
            tsb = tpool.tile([128, NSLOT * O], F16, tag="tsb_2")
            _drains(pts, tsb, 128)
            t2_tiles[bs] = tsb

        def combine(tw, slabs, q2t, dmas):
            prod = apool.tile([128, NQU * O], F16, tag="prod")
            pa = prod[:tw, :]
            qa = q2t[:tw, :]
            for (a, qb, t0, n) in AGROUPS:
                ts = slabs[a][:tw, :]
                in0 = AP(ts.tensor, ts.offset + t0 * O,
                         [ts.ap[0], [O, n], [2, O // 2], [1, 2]])
                in1 = AP(qa.tensor, qa.offset + 2 * qb,
                         [qa.ap[0], [2, n], [0, O // 2], [1, 2]])
                outp = AP(pa.tensor, pa.offset + qb * O,
                          [pa.ap[0], [O, n], [2, O // 2], [1, 2]])
                nc.vector.tensor_tensor(out=outp, in0=in0, in1=in1,
                                        op=mybir.AluOpType.mult)

            def fold(eng, src, n_el, dst):
                half = n_el // 2
                eng.tensor_tensor(
                    out=dst[:tw, 0:half], in0=src[:tw, 0:half],
                    in1=src[:tw, half:n_el], op=mybir.AluOpType.add)

            # tree over slots 0..79; slot 80 rides to the host as a carry
            h40 = rpool.tile([128, 40 * O], F16, tag="h40")
            fold(nc.vector, prod, 80 * O, h40)
            h20 = rpool.tile([128, 20 * O], F16, tag="h20")
            fold(nc.gpsimd, h40, 40 * O, h20)
            h10 = rpool.tile([128, 10 * O], F16, tag="h10")
            fold(nc.vector, h20, 20 * O, h10)
            h5 = rpool.tile([128, 5 * O], F16, tag="h5")
            fold(nc.vector, h10, 10 * O, h5)
            for (orow, plo, phi) in dmas:
                nc.sync.dma_start(outh_d[orow:orow + (phi - plo), :],
                                  h5[plo:phi, :])
                nc.sync.dma_start(outc_d[orow:orow + (phi - plo), :],
                                  prod[plo:phi, 80 * O:81 * O])

        # ---------------- prologue ----------------
        for fr in range(-1, 4):
            load_feat(fr)
        phase1(0)
        phase1(1)
        for ct in range(2):
            for rp in range(-2, 3):
                build_T(ct, rp)
        for bs in range(-2, 2):
            build_T2(bs)

        # ---------------- main loop ----------------
        for r in range(RS):
            if r <= RS - 3:
                load_feat(r + 3)
                phase1(r + 2)
            if r + 3 <= RS + 1:
                build_T(0, r + 3)
                build_T(1, r + 3)
            if r + 2 <= RS:
                build_T2(r + 2)
            for ct, (c0, tw) in enumerate(CT01):
                slabs = {a: t_tiles[ct][r + a] for a in range(-2, 3)}
                combine(tw, slabs, q2_tiles[(r, ct)],
                        [(r * W + (c0 - PADC), 0, tw)])
            pidx = None
            if r % 2 == 1 and r <= 43:
                pidx = (r - 1) // 2
            elif r == RS - 1:
                pidx = len(PAIRS) - 1
            if pidx is not None:
                ra = PAIRS[pidx][0]
                slabs = {a: t2_tiles[ra + a] for a in range(-2, 3)}
                combine(128, slabs, q2pair[pidx],
                        [(ra * W + 256, 0, 64), ((ra + 1) * W + 256, 64, 128)])

    nc.compile()
    return nc


# ------------------------- host side -------------------------

_nc_cache = [None]


def _get_nc():
    if _nc_cache[0] is None:
        _nc_cache[0] = build_module()
    return _nc_cache[0]


def _consts(weight, off_w, off_b):
    wk = np.zeros((NK, C, O), np.float32)
    for g in range(9):
        for og in range(8):
            for cg in range(8):
                for k in range(NK):
                    wk[k, g * 8 + cg, g * 8 + og] = weight[
                        g * 8 + og, cg, k // 3, k % 3]
    # wts columns follow the shift-major PSUM slot order
    wts = np.zeros((C, 36 * O), np.float16)
    for b, (s, slots) in enumerate(PBANKS):
        for w, ent in enumerate(slots):
            if ent is None:
                continue
            ky, i = ent
            _, kx, v = SLOT9[i]
            k = ky * 3 + kx
            m = b * 6 + w
            wts[:, m * O:(m + 1) * O] = wk[k].astype(np.float16)

    # offset conv fused with the one-hot tent selector
    cwy = np.zeros((C, 9 * NQR), np.float16)
    cwx = np.zeros((C, 9 * NQR), np.float16)
    for t in range(9):
        ow = off_w[:, :, t // 3, t % 3].T.astype(np.float32)  # [C, OC]
        cwy[:, t * NQR:(t + 1) * NQR] = (ow @ _REPY).astype(np.float16)
        cwx[:, t * NQR:(t + 1) * NQR] = (ow @ _REPX).astype(np.float16)
    biasu = (_BIASU + _REPY.T @ off_b).reshape(NQR, 1).astype(np.float32)
    biasv = (_BIASV + _REPX.T @ off_b).reshape(NQR, 1).astype(np.float32)

    dmat = np.zeros((NQR, 2 * NQR), np.float16)
    for j in range(NQR):
        dmat[j, 2 * j] = 1.0
        dmat[j, 2 * j + 1] = 1.0
    return {
        "wts": wts, "cwy": cwy, "cwx": cwx,
        "biasu": biasu, "biasv": biasv, "dmat": dmat,
    }


def _slab(x_b, halo, rows):
    out = []
    for q in range(NQ):
        s = np.zeros((C, rows, WP), np.float16)
        lo, hi = q * RS - halo, q * RS + RS + halo
        clo, chi = max(lo, 0), min(hi, H)
        s[:, clo - lo: clo - lo + (chi - clo), PADC:PADC + W] = x_b[:, clo:chi]
        out.append(np.ascontiguousarray(s.reshape(C, rows * WP)))
    return out


def _patch_outliers(out, inp, offset_feat, weight, off_w, off_b, tya, txa):
    """Fix (tap,pixel) events where |offset| >= 1 (center tent == 0)."""
    featp = np.pad(offset_feat, ((0, 0), (0, 0), (1, 1), (1, 1)))
    evs = []
    for k in range(NK):
        fy = tya[:, ROWY[k]] < 1e-3
        fx = txa[:, ROWX[k]] < 1e-3
        bs, ys, xs = np.nonzero(fy | fx)
        for b, y, x in zip(bs, ys, xs):
            evs.append((b, k, y, x))
    g = np.arange(O) // 8
    for (b, k, y, x) in evs:
        ky, kx = k // 3, k % 3
        nb = featp[b, :, y:y + 3, x:x + 3]
        dyv = float((off_w[2 * k] * nb).sum() + off_b[2 * k])
        dxv = float((off_w[2 * k + 1] * nb).sum() + off_b[2 * k + 1])
        py = y + ky - 1 + dyv
        px = x + kx - 1 + dxv
        y0, x0 = int(np.floor(py)), int(np.floor(px))
        wy1, wx1 = py - y0, px - x0
        s_true = np.zeros(C, np.float32)
        for yi, wy in ((y0, 1.0 - wy1), (y0 + 1, wy1)):
            for xi, wx in ((x0, 1.0 - wx1), (x0 + 1, wx1)):
                if 0 <= yi < H and 0 <= xi < W and wy * wx != 0.0:
                    s_true += (wy * wx) * inp[b, :, yi, xi]
        s_kern = np.zeros(C, np.float32)
        for u in (-1, 0, 1):
            tyv = max(0.0, 1.0 - abs(dyv - u))
            if tyv == 0.0:
                continue
            for v in (-1, 0, 1):
                txv = max(0.0, 1.0 - abs(dxv - v))
                if txv == 0.0:
                    continue
                yy, xx = y + ky - 1 + u, x + kx - 1 + v
                if 0 <= yy < H and 0 <= xx < W:
                    s_kern += (tyv * txv) * inp[b, :, yy, xx]
        delta = (s_true - s_kern).reshape(9, 8)
        corr = (weight[:, :, ky, kx] * delta[g]).sum(axis=1)
        out[b, :, y, x] += corr


def kernel(input, offset_feat, weight, off_w, off_b):
    input = np.asarray(input, np.float32)
    offset_feat = np.asarray(offset_feat, np.float32)
    weight = np.asarray(weight, np.float32)
    off_w = np.asarray(off_w, np.float32)
    off_b = np.asarray(off_b, np.float32)

    nc = _get_nc()
    consts = _consts(weight, off_w, off_b)
    in_maps = []
    for b in range(B):
        imgs = _slab(input[b], HALO, RSP)
        feats = _slab(offset_feat[b], 1, FROWS)
        for q in range(NQ):
            m = dict(consts)
            m["img"] = imgs[q]
            m["feat"] = feats[q]
            in_maps.append(m)

    res = bass_utils.run_bass_kernel_spmd(
        nc, in_maps, core_ids=list(range(N_CORES)))

    out = np.empty((B, O, H, W), np.float32)
    tya = np.empty((B, NQR, H, W), np.float32)
    txa = np.empty((B, NQR, H, W), np.float32)
    for ci in range(N_CORES):
        b, q = ci // NQ, ci % NQ
        oh = res.results[ci]["outh"].reshape(RS, W, 5, O).astype(np.float32)
        ocr = res.results[ci]["outc"].reshape(RS, W, O).astype(np.float32)
        o = oh.sum(axis=2) + ocr
        out[b, :, q * RS:(q + 1) * RS, :] = o.transpose(2, 0, 1)
        tya[b, :, q * RS:(q + 1) * RS, :] = (
            res.results[ci]["tya"].reshape(NQR, RS, W).astype(np.float32))
        txa[b, :, q * RS:(q + 1) * RS, :] = (
            res.results[ci]["txa"].reshape(NQR, RS, W).astype(np.float32))

    _patch_outliers(out, input, offset_feat, weight, off_w, off_b, tya, txa)
    return out


if __name__ == "__main__":
    import reference as ref
    inputs = {k: np.asarray(v) for k, v in ref.setup_inputs().items()}
    got = kernel(**inputs)
    print("out", got.shape, got.dtype)
